# revision 1
# baseline (speedup 1.0000x reference)
"""Trainium2 Bass kernel for the ConvModule problem.

Computes, for x (B=16, T=1024, C=512) fp32:
    h = LayerNorm_C(x) -> pw conv C->2C + Swish -> k=5 conv 2C->2C
      -> GLU -> BatchNorm(eval) -> pw conv C->C
Data-parallel over batch across 8 NeuronCores (2 batches/core, weights
replicated).  LN gamma/beta folded into w1/b1 and BN folded into w3/b3 on
the host, so the device only does: normalize, three matmul stages, Swish,
GLU.

Device data layout is [channel, time] (channels on partitions) for the
whole matmul chain; the final conv swaps matmul operands (activations as
the stationary lhsT) so its PSUM output lands directly in [time, channel]
DRAM layout.
"""

import os
from contextlib import ExitStack

import numpy as np

import concourse.bass as bass
import concourse.bacc as bacc
import concourse.tile as tile
from concourse import mybir
from concourse.masks import make_identity
from concourse.bass_utils import run_bass_kernel_spmd

B, T, C, K = 16, 1024, 512, 5
EPS_LN = 1e-5
EPS_BN = 1e-5
NCORES = 8
BLOC = B // NCORES          # batches per core
P = 128                     # SBUF partitions
CB = C // P                 # 4 channel blocks of the C dim
OB = (2 * C) // P           # 8 channel blocks of the 2C dim
TH = T // 2                 # 512: matmul moving-dim / PSUM-bank size
F32 = mybir.dt.float32

# matmul input dtype: bf16 runs the PE at 1 cycle/row; fp32 at 4.
MM_DT = mybir.dt.float32 if os.environ.get("KERNEL_FP32") else mybir.dt.bfloat16


def build_nc() -> bass.Bass:
    nc = bacc.Bacc("TRN2")

    xs = nc.declare_dram_parameter("xs", [BLOC, T, C], F32, isOutput=False)
    w1t = nc.declare_dram_parameter("w1t", [CB, P, 2 * C], MM_DT, isOutput=False)
    w2s = nc.declare_dram_parameter("w2s", [K, OB, P, 2 * C], MM_DT, isOutput=False)
    w3t = nc.declare_dram_parameter("w3t", [CB, P, C], MM_DT, isOutput=False)
    b1 = nc.declare_dram_parameter("b1", [P, OB], F32, isOutput=False)
    b2 = nc.declare_dram_parameter("b2", [P, OB], F32, isOutput=False)
    b3 = nc.declare_dram_parameter("b3", [P, C], F32, isOutput=False)
    out = nc.declare_dram_parameter("out", [BLOC, T, C], F32, isOutput=True)

    with ExitStack() as ctx:
        tc = ctx.enter_context(tile.TileContext(nc))

        consts = ctx.enter_context(tc.tile_pool(name="consts", bufs=1))
        xin = ctx.enter_context(tc.tile_pool(name="xin", bufs=4))
        stats = ctx.enter_context(tc.tile_pool(name="stats", bufs=4))
        hNp = ctx.enter_context(tc.tile_pool(name="hNp", bufs=2))
        sigp = ctx.enter_context(tc.tile_pool(name="sigp", bufs=1))
        xbigp = ctx.enter_context(tc.tile_pool(name="xbigp", bufs=2))
        outp = ctx.enter_context(tc.tile_pool(name="outp", bufs=1))
        tp_psum = ctx.enter_context(tc.tile_pool(name="tp_psum", bufs=2, space="PSUM"))
        mm_psum = ctx.enter_context(tc.tile_pool(name="mm_psum", bufs=4, space="PSUM"))
        o_psum = ctx.enter_context(tc.tile_pool(name="o_psum", bufs=2, space="PSUM"))

        # ---- constants / weights (loaded once) ----
        ident = consts.tile([P, P], MM_DT, tag="ident")
        make_identity(nc, ident)
        epssb = consts.tile([P, 1], F32, tag="eps")
        nc.vector.memset(epssb, EPS_LN)
        b1sb = consts.tile([P, OB], F32, tag="b1")
        nc.sync.dma_start(out=b1sb, in_=b1[:])
        b2sb = consts.tile([P, OB], F32, tag="b2")
        nc.sync.dma_start(out=b2sb, in_=b2[:])
        b3sb = consts.tile([P, C], F32, tag="b3")
        nc.sync.dma_start(out=b3sb, in_=b3[:])
        w1sb = []
        for cb in range(CB):
            w = consts.tile([P, 2 * C], MM_DT, tag=f"w1_{cb}", name=f"w1_{cb}")
            nc.sync.dma_start(out=w, in_=w1t[cb])
            w1sb.append(w)
        w3sb = []
        for cb in range(CB):
            w = consts.tile([P, C], MM_DT, tag=f"w3_{cb}", name=f"w3_{cb}")
            nc.sync.dma_start(out=w, in_=w3t[cb])
            w3sb.append(w)
        w2sb = {}
        for k in range(K):
            for ib in range(OB):
                w = consts.tile([P, 2 * C], MM_DT, tag=f"w2_{k}_{ib}", name=f"w2_{k}_{ib}")
                nc.sync.dma_start(out=w, in_=w2s[k, ib])
                w2sb[(k, ib)] = w

        # Persistent activation tiles (reused across both batches).
        # h1 is the Swish output, zero-padded by 2 columns on each side so
        # the k=5 conv can slide its window without edge cases.
        h1 = []
        for ib in range(OB):
            t_ = consts.tile([P, T + 4], MM_DT, tag=f"h1_{ib}", name=f"h1_{ib}")
            nc.vector.memset(t_[:, 0:2], 0.0)
            nc.vector.memset(t_[:, T + 2 : T + 4], 0.0)
            h1.append(t_)
        hG = [consts.tile([P, T], MM_DT, tag=f"hG_{cb}", name=f"hG_{cb}") for cb in range(CB)]

        for b in range(BLOC):
            # ---- Phase A: LayerNorm ([tok, C] tiles) + PE transpose to
            # hN laid out [c-within-block, cb*T + t] ----
            # One big x DMA per batch: fresh pool slot + fresh SWDGE queue
            # keeps the DMA instruction at <=1 sync wait (walrus limit).
            xbig = xbigp.tile([P, T // P, C], F32, tag="xbig")
            nc.gpsimd.dma_start(
                out=xbig, in_=xs[b].rearrange("(tb p) c -> p tb c", p=P)
            )
            hN = hNp.tile([P, CB * T], MM_DT, tag="hN")
            hN3 = hN[:, :].rearrange("p (c t) -> p c t", c=CB)
            for tb in range(T // P):
                xt = xbig[:, tb, :]
                st6 = stats.tile([P, 6], F32, tag="st6")
                nc.vector.bn_stats(out=st6, in_=xt)
                mv = stats.tile([P, 2], F32, tag="mv")
                nc.vector.bn_aggr(out=mv, in_=st6)
                rstd = stats.tile([P, 1], F32, tag="rstd")
                nc.scalar.activation(
                    out=rstd, in_=mv[:, 1:2],
                    func=mybir.ActivationFunctionType.Sqrt,
                    bias=epssb, scale=1.0,
                )
                nc.vector.reciprocal(out=rstd, in_=rstd)
                xn = xin.tile([P, C], MM_DT, tag="xn")
                nc.vector.tensor_scalar(
                    out=xn, in0=xt,
                    scalar1=mv[:, 0:1], scalar2=rstd,
                    op0=mybir.AluOpType.subtract, op1=mybir.AluOpType.mult,
                )
                ps = tp_psum.tile([P, CB * P], MM_DT, tag="tp")
                for cb in range(CB):
                    nc.tensor.transpose(
                        ps[:, cb * P : (cb + 1) * P],
                        xn[:, cb * P : (cb + 1) * P],
                        ident,
                    )
                nc.scalar.copy(
                    out=hN3[:, :, tb * P : (tb + 1) * P],
                    in_=ps[:, :].rearrange("p (c i) -> p c i", c=CB),
                )

            # ---- Phase B: pointwise conv C->2C + Swish(psum + b1) ----
            for ob in range(OB):
                pA = mm_psum.tile([P, TH], F32, tag="mm")
                pB = mm_psum.tile([P, TH], F32, tag="mm")
                for cb in range(CB):
                    w = w1sb[cb][:, ob * P : (ob + 1) * P]
                    st, sp = cb == 0, cb == CB - 1
                    nc.tensor.matmul(pA, w, hN3[:, cb, 0:TH], start=st, stop=sp)
                    nc.tensor.matmul(pB, w, hN3[:, cb, TH:T], start=st, stop=sp)
                # Swish(z) = z * sigmoid(z), z = psum + b1
                for ph, psum, lo in ((0, pA, 2), (1, pB, 2 + TH)):
                    sg = sigp.tile([P, TH], MM_DT, tag=f"sw{ph}", name=f"sw{ph}")
                    nc.scalar.activation(
                        out=sg, in_=psum,
                        func=mybir.ActivationFunctionType.Sigmoid,
                        bias=b1sb[:, ob : ob + 1], scale=1.0,
                    )
                    z = sigp.tile([P, TH], MM_DT, tag=f"z{ph}", name=f"z{ph}")
                    nc.vector.tensor_scalar_add(
                        out=z, in0=psum, scalar1=b1sb[:, ob : ob + 1]
                    )
                    nc.vector.tensor_mul(
                        out=h1[ob][:, lo : lo + TH], in0=z, in1=sg
                    )

            # ---- Phase C: k=5 conv 2C->2C + GLU ----
            # Gate halves (ob 4..7) first so their sigmoids are ready when
            # the value halves (ob 0..3) drain.
            sig = {}
            for ob in [4, 5, 6, 7, 0, 1, 2, 3]:
                pA = mm_psum.tile([P, TH], F32, tag="mm")
                pB = mm_psum.tile([P, TH], F32, tag="mm")
                first = True
                for k in range(K):
                    for ib in range(OB):
                        w = w2sb[(k, ib)][:, ob * P : (ob + 1) * P]
                        last = (k == K - 1) and (ib == OB - 1)
                        nc.tensor.matmul(
                            pA, w, h1[ib][:, k : k + TH], start=first, stop=last
                        )
                        nc.tensor.matmul(
                            pB, w, h1[ib][:, TH + k : TH + k + TH],
                            start=first, stop=last,
                        )
                        first = False
                if ob >= 4:
                    j = ob - 4
                    sA = sigp.tile([P, TH], MM_DT, tag=f"sig{j}a", name=f"sig{j}a")
                    sB = sigp.tile([P, TH], MM_DT, tag=f"sig{j}b", name=f"sig{j}b")
                    nc.scalar.activation(
                        out=sA, in_=pA,
                        func=mybir.ActivationFunctionType.Sigmoid,
                        bias=b2sb[:, ob : ob + 1], scale=1.0,
                    )
                    nc.scalar.activation(
                        out=sB, in_=pB,
                        func=mybir.ActivationFunctionType.Sigmoid,
                        bias=b2sb[:, ob : ob + 1], scale=1.0,
                    )
                    sig[j] = (sA, sB)
                else:
                    j = ob
                    aA = sigp.tile([P, TH], MM_DT, tag=f"a{j}a", name=f"a{j}a")
                    aB = sigp.tile([P, TH], MM_DT, tag=f"a{j}b", name=f"a{j}b")
                    nc.vector.tensor_scalar_add(
                        out=aA, in0=pA, scalar1=b2sb[:, ob : ob + 1]
                    )
                    nc.vector.tensor_scalar_add(
                        out=aB, in0=pB, scalar1=b2sb[:, ob : ob + 1]
                    )
                    sA, sB = sig[j]
                    nc.vector.tensor_mul(out=hG[j][:, 0:TH], in0=aA, in1=sA)
                    nc.vector.tensor_mul(out=hG[j][:, TH:T], in0=aB, in1=sB)

            # ---- Phase D: pointwise conv C->C (+ BN fold) + bias ----
            # lhsT = activations so PSUM comes out [t, o] = DRAM layout.
            obig = outp.tile([P, T // P, C], F32, tag="obig")
            for tb in range(T // P):
                po = o_psum.tile([P, C], F32, tag="po")
                for cb in range(CB):
                    nc.tensor.matmul(
                        po,
                        hG[cb][:, tb * P : (tb + 1) * P],
                        w3sb[cb],
                        start=(cb == 0), stop=(cb == CB - 1),
                    )
                nc.vector.tensor_add(out=obig[:, tb, :], in0=po, in1=b3sb)
            nc.gpsimd.dma_start(
                out=out[b].rearrange("(tb p) c -> p tb c", p=P), in_=obig
            )

    nc.compile()
    return nc


def prepare_inputs(x, ln_g, ln_b, w1, b1, w2, b2, bn_g, bn_b, bn_mean, bn_var, w3, b3):
    """Host-side folding + layout. Returns per-core input maps."""
    f = np.float32
    x = np.asarray(x, f)
    ln_g, ln_b = np.asarray(ln_g, f), np.asarray(ln_b, f)
    w1, b1 = np.asarray(w1, f), np.asarray(b1, f)
    w2, b2 = np.asarray(w2, f), np.asarray(b2, f)
    bn_g, bn_b = np.asarray(bn_g, f), np.asarray(bn_b, f)
    bn_mean, bn_var = np.asarray(bn_mean, f), np.asarray(bn_var, f)
    w3, b3 = np.asarray(w3, f), np.asarray(b3, f)

    # Fold LN affine into conv1, BN (eval) into conv3.
    w1f = w1 * ln_g[None, :]
    b1f = b1 + w1 @ ln_b
    s_bn = bn_g / np.sqrt(bn_var + EPS_BN)
    w3f = w3 * s_bn[None, :]
    b3f = b3 + w3 @ (bn_b - bn_mean * s_bn)

    mdt = mybir.dt.np(MM_DT)
    w1t = np.ascontiguousarray(w1f.T.reshape(CB, P, 2 * C)).astype(mdt)
    w2s = np.ascontiguousarray(w2.reshape(K, OB, P, 2 * C)).astype(mdt)
    w3t = np.ascontiguousarray(w3f.T.reshape(CB, P, C)).astype(mdt)
    b1d = np.ascontiguousarray(b1f.reshape(OB, P).T)
    b2d = np.ascontiguousarray(b2.reshape(OB, P).T)
    b3d = np.ascontiguousarray(np.broadcast_to(b3f, (P, C)))

    shared = {"w1t": w1t, "w2s": w2s, "w3t": w3t, "b1": b1d, "b2": b2d, "b3": b3d}
    in_maps = []
    for c in range(NCORES):
        m = dict(shared)
        m["xs"] = np.ascontiguousarray(x[c * BLOC : (c + 1) * BLOC])
        in_maps.append(m)
    return in_maps


_NC = None
LAST_RESULTS = None


def kernel(**inputs) -> np.ndarray:
    global _NC, LAST_RESULTS
    if _NC is None:
        _NC = build_nc()
    in_maps = prepare_inputs(**inputs)
    res = run_bass_kernel_spmd(_NC, in_maps, list(range(NCORES)))
    LAST_RESULTS = res
    return np.concatenate([r["out"] for r in res.results], axis=0)



# revision 21
# speedup vs baseline: 1.0263x; 1.0263x over previous
"""Trainium2 Bass kernel for the ConvModule problem (DFT8 conv version).

Computes, for x (B=16, T=1024, C=512) fp32:
    h = LayerNorm_C(x) -> pw conv C->2C + Swish -> k=5 conv 2C->2C
      -> GLU -> BatchNorm(eval) -> pw conv C->C
Data-parallel over batch across 8 NeuronCores (2 batches/core, weights
replicated).  LN gamma/beta folded into w1/b1, BN folded into w3/b3 on the
host.

The k=5 'same' conv is computed as a length-8 cyclic correlation per tile of
4 outputs (exact since 3+4 <= 7), via a real FFT8 on the device (DVE/GpSimd
butterflies on stride-1 deinterleaved planes) and host-side transformed
weights U = conj(FFT8(w2 zero-padded))/8.  Per complex point j, three U
planes (Re, -Im, +Im) are stored so every PSUM contribution is a plain
accumulate:
    M_jr = A_j V_jr + B_j V_ji ,  M_ji = C_j V_jr + A_j V_ji
with A=Re(U), B=-Im(U), C=+Im(U).  This needs 14 GEMM passes per 4 outputs
instead of the direct method's 20, cutting Tensor-engine time ~1.4x.
"""

from contextlib import ExitStack

import numpy as np

import concourse.bass as bass
import concourse.bacc as bacc
import concourse.tile as tile
from concourse import mybir
from concourse.masks import make_identity
from concourse.bass_utils import run_bass_kernel_spmd

B, T, C, K = 16, 1024, 512, 5
EPS_LN = 1e-5
EPS_BN = 1e-5
NCORES = 8
BLOC = B // NCORES          # batches per core
P = 128                     # SBUF partitions
CB = C // P                 # 4 channel blocks of the C dim
OB = (2 * C) // P           # 8 channel blocks of the 2C dim
TH = T // 2                 # 512
NT = T // 4                 # 256 conv tiles per batch (4 outputs each)
NU = 11                     # stored U planes: U0, U4, (A,B,C) x j=1..3
F32 = mybir.dt.float32
BF16 = mybir.dt.bfloat16
RS2 = float(1.0 / np.sqrt(2.0))
SQ2 = float(np.sqrt(2.0))

AF = mybir.ActivationFunctionType
ALU = mybir.AluOpType

# GEMM pass lists: (psum plane index, [(u_idx, v_name), ...])
# u planes: 0:U0 1:U4 2:A1 3:B1 4:C1 5:A2 6:B2 7:C2 8:A3 9:B3 10:C3
# E group: M0, M4, M2r, M2i ; O group: M1r, M1i, M3r, M3i
MPASS_E = [
    (0, [(0, "v0")]),
    (1, [(1, "v4")]),
    (2, [(5, "v2r"), (6, "v2i")]),
    (3, [(7, "v2r"), (5, "v2i")]),
]
MPASS_O = [
    (0, [(2, "v1r"), (3, "v1i")]),
    (1, [(4, "v1r"), (2, "v1i")]),
    (2, [(8, "v3r"), (9, "v3i")]),
    (3, [(10, "v3r"), (8, "v3i")]),
]
VNAMES = ["v0", "v4", "v1r", "v1i", "v2r", "v2i", "v3r", "v3i"]


def build_nc() -> bass.Bass:
    nc = bacc.Bacc("TRN2")

    xs = nc.declare_dram_parameter("xs", [BLOC, T, C], BF16, isOutput=False)
    w1t = nc.declare_dram_parameter("w1t", [CB, P, 2 * C], BF16, isOutput=False)
    ut = nc.declare_dram_parameter("ut", [OB, P, NU, OB, P], BF16, isOutput=False)
    w3t = nc.declare_dram_parameter("w3t", [CB, P, C], BF16, isOutput=False)
    b1 = nc.declare_dram_parameter("b1", [P, OB], F32, isOutput=False)
    b2 = nc.declare_dram_parameter("b2", [P, OB], F32, isOutput=False)
    b3 = nc.declare_dram_parameter("b3", [P, C], F32, isOutput=False)
    out = nc.declare_dram_parameter("out", [BLOC, T, C], F32, isOutput=True)

    with ExitStack() as ctx:
        tc = ctx.enter_context(tile.TileContext(nc))

        consts = ctx.enter_context(tc.tile_pool(name="consts", bufs=1))
        xin = ctx.enter_context(tc.tile_pool(name="xin", bufs=2))
        stats = ctx.enter_context(tc.tile_pool(name="stats", bufs=4))
        hNp = ctx.enter_context(tc.tile_pool(name="hNp", bufs=1))
        h1rp = ctx.enter_context(tc.tile_pool(name="h1rp", bufs=4))
        fsc = ctx.enter_context(tc.tile_pool(name="fsc", bufs=1))
        vpool = ctx.enter_context(tc.tile_pool(name="vpool", bufs=1))
        upool = ctx.enter_context(tc.tile_pool(name="upool", bufs=2))
        mcp = ctx.enter_context(tc.tile_pool(name="mcp", bufs=2))
        isc = ctx.enter_context(tc.tile_pool(name="isc", bufs=1))
        yap = ctx.enter_context(tc.tile_pool(name="yap", bufs=2))
        hGp = ctx.enter_context(tc.tile_pool(name="hGp", bufs=1))
        outp = ctx.enter_context(tc.tile_pool(name="outp", bufs=2))
        cv_psum = ctx.enter_context(tc.tile_pool(name="cv_psum", bufs=2, space="PSUM"))
        ab_psum = ctx.enter_context(tc.tile_pool(name="ab_psum", bufs=2, space="PSUM"))
        o_psum = ctx.enter_context(tc.tile_pool(name="o_psum", bufs=2, space="PSUM"))

        # ---- constants / persistent weights ----
        ident = consts.tile([P, P], BF16, tag="ident")
        make_identity(nc, ident)
        epssb = consts.tile([P, 1], F32, tag="eps")
        nc.vector.memset(epssb, EPS_LN)
        b1sb = consts.tile([P, OB], F32, tag="b1")
        nc.sync.dma_start(out=b1sb, in_=b1[:])
        b2sb = consts.tile([P, OB], F32, tag="b2")
        nc.sync.dma_start(out=b2sb, in_=b2[:])
        b3sb = consts.tile([P, C], F32, tag="b3")
        nc.sync.dma_start(out=b3sb, in_=b3[:])
        w1sb = []
        for cb in range(CB):
            w = consts.tile([P, 2 * C], BF16, tag=f"w1_{cb}", name=f"w1_{cb}")
            nc.sync.dma_start(out=w, in_=w1t[cb])
            w1sb.append(w)
        w3sb = []
        for cb in range(CB):
            w = consts.tile([P, C], BF16, tag=f"w3_{cb}", name=f"w3_{cb}")
            nc.sync.dma_start(out=w, in_=w3t[cb])
            w3sb.append(w)

        # V planes: [128, BLOC, NT] bf16 per (plane, ib); halves written per batch
        vsb = {}
        for vn in VNAMES:
            for ib in range(OB):
                v = vpool.tile([P, BLOC, NT], BF16, tag=f"{vn}_{ib}",
                               name=f"{vn}_{ib}")
                vsb[(vn, ib)] = v

        hg_tiles = {}
        hN_all = {}

        # ---------- Phase A: LN + transpose to [c, t] ----------
        def phase_A(b):
            xh = []
            xsr = xs[b].rearrange("(h tb p) c -> h p tb c", h=4, p=P)
            for h in range(4):
                xb = xin.tile([P, T // P // 4, C], BF16, tag="xbig",
                              name=f"xbig_{b}_{h}")
                nc.gpsimd.dma_start(out=xb, in_=xsr[h])
                xh.append(xb)
            hN = hNp.tile([P, CB * T], BF16, tag="hN", name=f"hN_{b}")
            hN_all[b] = hN
            hN3 = hN[:, :].rearrange("p (c t) -> p c t", c=CB)
            for tb in range(T // P):
                xt = xh[tb // 2][:, tb % 2, :]
                st6 = stats.tile([P, 6], F32, tag="st6")
                nc.vector.bn_stats(out=st6, in_=xt)
                mv = stats.tile([P, 2], F32, tag="mv")
                nc.vector.bn_aggr(out=mv, in_=st6)
                rstd = stats.tile([P, 1], F32, tag="rstd")
                nc.scalar.activation(
                    out=rstd, in_=mv[:, 1:2], func=AF.Sqrt,
                    bias=epssb, scale=1.0,
                )
                nc.vector.reciprocal(out=rstd, in_=rstd)
                nmu = stats.tile([P, 1], F32, tag="nmu")
                nc.vector.tensor_scalar(
                    out=nmu, in0=mv[:, 0:1], scalar1=rstd, scalar2=-1.0,
                    op0=ALU.mult, op1=ALU.mult,
                )
                xn = stats.tile([P, C], BF16, tag="xn")
                nc.scalar.activation(
                    out=xn, in_=xt, func=AF.Identity, bias=nmu, scale=rstd,
                )
                ps = ab_psum.tile([P, TH], BF16, tag="ab", name=f"tp_{b}_{tb}")
                for cb in range(CB):
                    nc.tensor.transpose(
                        ps[:, cb * P:(cb + 1) * P], xn[:, cb * P:(cb + 1) * P],
                        ident,
                    )
                nc.scalar.copy(
                    out=hN3[:, :, tb * P:(tb + 1) * P],
                    in_=ps[:, 0:CB * P].rearrange("p (c i) -> p c i", c=CB),
                )

        # ---------- Phase B: conv1 C->2C + Swish into deinterleaved planes ----
        def phase_B(b):
            hN3 = hN_all[b][:, :].rearrange("p (c t) -> p c t", c=CB)
            h1r = []
            for ob in range(OB):
                t_ = h1rp.tile([P, 4, NT + 2], BF16, tag="h1r",
                               name=f"h1r_{ob}_{b}")
                nc.vector.memset(t_[:, :, 0:1], 0.0)
                nc.vector.memset(t_[:, :, NT + 1:NT + 2], 0.0)
                # Silu(z + b1); r_j[u] = h1[4u+j] at storage col u+1
                for ph in range(2):
                    pz = ab_psum.tile([P, TH], F32, tag="ab",
                                      name=f"pz_{ob}_{b}_{ph}")
                    for cb in range(CB):
                        w = w1sb[cb][:, ob * P:(ob + 1) * P]
                        nc.tensor.matmul(
                            pz, w, hN3[:, cb, ph * TH:(ph + 1) * TH],
                            start=(cb == 0), stop=(cb == CB - 1))
                    dst = t_[:, :, 128 * ph + 1:128 * ph + 129]
                    nc.scalar.activation(
                        out=dst.rearrange("p j u -> p u j"),
                        in_=pz,
                        func=AF.Silu, bias=b1sb[:, ob:ob + 1], scale=1.0,
                    )
                h1r.append(t_)
            return h1r

        # ---------- FFT8 per (ib, batch): h1r -> 8 V planes ----------
        def fft_ib(r, ib, b):
            # d_i views of the deinterleaved planes, all [P, NT], stride 1
            d = [
                r[:, 2, 0:NT], r[:, 3, 0:NT], r[:, 0, 1:NT + 1],
                r[:, 1, 1:NT + 1], r[:, 2, 1:NT + 1], r[:, 3, 1:NT + 1],
                r[:, 0, 2:NT + 2], r[:, 1, 2:NT + 2],
            ]

            def tt(engine, out, in0, in1, op):
                engine.scalar_tensor_tensor(
                    out=out, in0=in0, scalar=1.0, in1=in1,
                    op0=ALU.mult, op1=op,
                )

            s, t_ = [], []
            for i in range(4):
                si = fsc.tile([P, NT], BF16, tag=f"s{i}", name=f"s{i}_{ib}_{b}")
                tt(nc.vector, si, d[i], d[i + 4], ALU.add)
                s.append(si)
                ti = fsc.tile([P, NT], BF16, tag=f"t{i}", name=f"t{i}_{ib}_{b}")
                tt(nc.vector, ti, d[i], d[i + 4], ALU.subtract)
                t_.append(ti)
            u0 = fsc.tile([P, NT], BF16, tag="u0", name=f"u0_{ib}_{b}")
            tt(nc.vector, u0, s[0], s[2], ALU.add)
            u1 = fsc.tile([P, NT], BF16, tag="u1", name=f"u1_{ib}_{b}")
            tt(nc.vector, u1, s[1], s[3], ALU.add)
            V = {vn: vsb[(vn, ib)][:, b, :] for vn in VNAMES}
            tt(nc.vector, V["v0"], u0, u1, ALU.add)
            tt(nc.vector, V["v4"], u0, u1, ALU.subtract)
            tt(nc.vector, V["v2r"], s[0], s[2], ALU.subtract)
            tt(nc.vector, V["v2i"], s[3], s[1], ALU.subtract)
            a = fsc.tile([P, NT], BF16, tag="fa", name=f"fa_{ib}_{b}")
            tt(nc.vector, a, t_[1], t_[3], ALU.subtract)
            bb = fsc.tile([P, NT], BF16, tag="fb", name=f"fb_{ib}_{b}")
            tt(nc.vector, bb, t_[1], t_[3], ALU.add)
            nc.vector.scalar_tensor_tensor(
                out=V["v1r"], in0=a, scalar=RS2, in1=t_[0],
                op0=ALU.mult, op1=ALU.add)
            nc.vector.scalar_tensor_tensor(
                out=V["v3r"], in0=a, scalar=-RS2, in1=t_[0],
                op0=ALU.mult, op1=ALU.add)
            # V1i = -t2 - RS2*b = (b*-RS2) - t2 ; V3i = t2 - RS2*b = (b*-RS2) + t2
            nc.vector.scalar_tensor_tensor(
                out=V["v1i"], in0=bb, scalar=-RS2, in1=t_[2],
                op0=ALU.mult, op1=ALU.subtract)
            nc.vector.scalar_tensor_tensor(
                out=V["v3i"], in0=bb, scalar=-RS2, in1=t_[2],
                op0=ALU.mult, op1=ALU.add)

        # ---------- conv GEMM + IFFT per (ob, batch) ----------
        def gemm_group(mm, passes, b, usb):
            for pl, plist in passes:
                n = len(plist) * OB
                i = 0
                for (ui, vn) in plist:
                    for ib in range(OB):
                        nc.tensor.matmul(
                            mm[:, pl, :],
                            usb[:, ui, ib, :],
                            vsb[(vn, ib)][:, b, :],
                            start=(i == 0), stop=(i == n - 1),
                        )
                        i += 1

        def conv_ob(ob, b, usb):
            def stt(out_, in0, sc, in1, engine=None):
                (engine or nc.vector).scalar_tensor_tensor(
                    out=out_, in0=in0, scalar=sc, in1=in1,
                    op0=ALU.mult, op1=ALU.add)

            def tl(tag):
                return isc.tile([P, NT], BF16, tag=tag, name=f"{tag}_{ob}_{b}")

            mc = mcp.tile([P, 8, NT], BF16, tag="mc", name=f"mc_{ob}_{b}")
            # E group: M0, M4, M2r, M2i
            mmE = cv_psum.tile([P, 4, NT], F32, tag="mm", name=f"mmE_{ob}_{b}")
            gemm_group(mmE, MPASS_E, b, usb)
            for pl in range(4):
                nc.scalar.copy(out=mc[:, pl, :], in_=mmE[:, pl, :])
            M0, M4 = mc[:, 0, :], mc[:, 1, :]
            M2r, M2i = mc[:, 2, :], mc[:, 3, :]
            Pt, Qt = tl("iP"), tl("iQ")
            stt(Pt, M4, 1.0, M0)
            stt(Qt, M4, -1.0, M0)
            E = []
            for k, (msrc, sc, base) in enumerate(
                    ((M2r, 2.0, Pt), (M2i, -2.0, Qt),
                     (M2r, -2.0, Pt), (M2i, 2.0, Qt))):
                e = tl(f"iE{k}")
                stt(e, msrc, sc, base)
                E.append(e)
            # O group: M1r, M1i, M3r, M3i
            mmO = cv_psum.tile([P, 4, NT], F32, tag="mm", name=f"mmO_{ob}_{b}")
            gemm_group(mmO, MPASS_O, b, usb)
            for pl in range(4):
                nc.scalar.copy(out=mc[:, 4 + pl, :], in_=mmO[:, pl, :])
            M1r, M1i = mc[:, 4, :], mc[:, 5, :]
            M3r, M3i = mc[:, 6, :], mc[:, 7, :]
            # y stored t-sequential; y_t written as stride-4 slices
            yt = yap.tile([P, T], BF16, tag="ya" if ob < CB else "yg",
                          name=f"y_{ob}_{b}")
            y = yt.rearrange("p (u j) -> p j u", j=4)
            w0 = tl("iw0")
            stt(w0, M3r, 1.0, M1r)
            stt(y[:, 0, :], w0, 2.0, E[0])
            aa, bb2, e1 = tl("iaa"), tl("ibb"), tl("ie1")
            stt(aa, M1i, -1.0, M1r)          # M1r - M1i
            stt(bb2, M3i, 1.0, M3r)          # M3r + M3i
            stt(e1, bb2, -1.0, aa)           # aa - bb2
            stt(y[:, 1, :], e1, SQ2, E[1])
            w2_ = tl("iw2")
            stt(w2_, M1i, -1.0, M3i)         # M3i - M1i
            stt(y[:, 2, :], w2_, 2.0, E[2])
            cc, dd, e3 = tl("icc"), tl("idd"), tl("ie3")
            stt(cc, M1i, 1.0, M1r)           # M1r + M1i
            stt(dd, M3i, -1.0, M3r)          # M3r - M3i
            stt(e3, dd, -1.0, cc)            # cc - dd
            stt(y[:, 3, :], e3, -SQ2, E[3])
            return yt

        # ---------- GLU per (value-ob v, batch) ----------
        def glu(v, b, ya, yg):
            sg = isc.tile([P, T], BF16, tag="sg", name=f"sg_{v}_{b}", bufs=2)
            nc.scalar.activation(
                out=sg, in_=yg,
                func=AF.Sigmoid, bias=b2sb[:, v + CB:v + CB + 1], scale=1.0,
            )
            hg = hGp.tile([P, T], BF16, tag=f"hg{v}", name=f"hg{v}_{b}")
            hg_tiles[(v, b)] = hg
            # hg = (ya + b2[v]) * sg
            nc.vector.scalar_tensor_tensor(
                out=hg, in0=ya, scalar=b2sb[:, v:v + 1], in1=sg,
                op0=ALU.add, op1=ALU.mult,
            )

        # ---------- Phase D: conv3 with activations stationary ----------
        def phase_D(b):
            for tb in range(T // P):
                po = o_psum.tile([P, C], F32, tag="po", name=f"po_{b}_{tb}")
                for cb in range(CB):
                    hg = hg_tiles[(cb, b)]
                    nc.tensor.matmul(
                        po, hg[:, P * tb:P * (tb + 1)], w3sb[cb],
                        start=(cb == 0), stop=(cb == CB - 1),
                    )
                obig = outp.tile([P, C], F32, tag="obig", name=f"ob_{b}_{tb}")
                nc.vector.tensor_add(out=obig, in0=po, in1=b3sb)
                nc.gpsimd.dma_start(
                    out=out[b].rearrange("(tb p) c -> p tb c", p=P)[:, tb, :],
                    in_=obig,
                )

        # ================= schedule =================
        for b in range(BLOC):
            phase_A(b)
            h1r = phase_B(b)
            for ib in range(OB):
                fft_ib(h1r[ib], ib, b)
        OBORDER = [0, CB, 1, 1 + CB, 2, 2 + CB, 3, 3 + CB]
        for b in range(BLOC):
            ya_cur = {}
            for ob in OBORDER:
                usb = upool.tile([P, NU, OB, P], BF16, tag="uslab",
                                 name=f"uslab_{ob}_{b}")
                nc.sync.dma_start(out=usb, in_=ut[ob])
                y = conv_ob(ob, b, usb)
                if ob < CB:
                    ya_cur[ob] = y
                else:
                    glu(ob - CB, b, ya_cur.pop(ob - CB), y)
            phase_D(b)

    nc.compile()
    return nc


def prepare_inputs(x, ln_g, ln_b, w1, b1, w2, b2, bn_g, bn_b, bn_mean, bn_var, w3, b3):
    """Host-side folding + DFT weight transform + layout."""
    f = np.float32
    bf = mybir.dt.np(BF16)
    x = np.asarray(x, f)
    ln_g, ln_b = np.asarray(ln_g, f), np.asarray(ln_b, f)
    w1, b1 = np.asarray(w1, f), np.asarray(b1, f)
    w2, b2 = np.asarray(w2, f), np.asarray(b2, f)
    bn_g, bn_b = np.asarray(bn_g, f), np.asarray(bn_b, f)
    bn_mean, bn_var = np.asarray(bn_mean, f), np.asarray(bn_var, f)
    w3, b3 = np.asarray(w3, f), np.asarray(b3, f)

    # Fold LN affine into conv1, BN (eval) into conv3.
    w1f = w1 * ln_g[None, :]
    b1f = b1 + w1 @ ln_b
    s_bn = bn_g / np.sqrt(bn_var + EPS_BN)
    w3f = w3 * s_bn[None, :]
    b3f = b3 + w3 @ (bn_b - bn_mean * s_bn)

    w1d = np.ascontiguousarray(w1f.T.reshape(CB, P, 2 * C)).astype(bf)
    w3d = np.ascontiguousarray(w3f.T.reshape(CB, P, C)).astype(bf)

    # U planes: Uc = conj(FFT8(pad(w2)))/8, w2 is (K, I, O)
    wf = np.fft.fft(np.pad(w2.astype(np.float64), ((0, 8 - K), (0, 0), (0, 0))),
                    axis=0)
    Uc = np.conj(wf) / 8.0
    planes = [Uc[0].real, Uc[4].real]
    for j in (1, 2, 3):
        planes += [Uc[j].real, -Uc[j].imag, Uc[j].imag]
    ud = np.stack(planes)                      # (NU, 2C_in, 2C_out)
    ud = ud.reshape(NU, OB, P, OB, P)          # (u, ib, p, ob, o)
    ud = np.ascontiguousarray(ud.transpose(3, 2, 0, 1, 4))  # (ob, p, u, ib, o)
    ud = ud.astype(bf)

    b1d = np.ascontiguousarray(b1f.reshape(OB, P).T)
    b2d = np.ascontiguousarray(b2.reshape(OB, P).T)
    b3d = np.ascontiguousarray(np.broadcast_to(b3f, (P, C)))

    shared = {"w1t": w1d, "ut": ud, "w3t": w3d, "b1": b1d, "b2": b2d, "b3": b3d}
    in_maps = []
    for c in range(NCORES):
        m = dict(shared)
        m["xs"] = np.ascontiguousarray(x[c * BLOC:(c + 1) * BLOC]).astype(bf)
        in_maps.append(m)
    return in_maps


_NC = None
LAST_RESULTS = None


def kernel(**inputs) -> np.ndarray:
    global _NC, LAST_RESULTS
    if _NC is None:
        _NC = build_nc()
    in_maps = prepare_inputs(**inputs)
    res = run_bass_kernel_spmd(_NC, in_maps, list(range(NCORES)))
    LAST_RESULTS = res
    return np.concatenate([r["out"] for r in res.results], axis=0)
